# revision 30
# baseline (speedup 1.0000x reference)
"""BinaryDense kernel for Trainium2: out = sign(x) @ sign(w).

x: [8192, 2048] f32, w: [2048, 2048] f32 -> out: [8192, 2048] f32.

Strategy: data-parallel shard of the batch dim across 8 NeuronCores
(1024 rows each, w replicated). The host hands each core its x shard
pre-transposed (layout choice for the shard) and both inputs as bf16
(sign-preserving for every magnitude the f32 inputs can contain --
bf16 underflows at ~1e-38; binarization itself stays on-device), which
halves input DMA to 12MB/core. Per core:
  - w streamed in (pass, k-sub-tile) 0.5MB slices, binarized on DVE to
    fp8e4 (+-0.5 via one tensor_scalar: (w >= 0) - 0.5) into separate
    small resident tiles (separate tiles => fine-grained scheduler
    dependencies, so matmuls start as soon as their slice lands).
  - x^T streamed in m-pair column groups [128, 16, 256], binarized the
    same way.
  - Matmuls in fp8 DoubleRow mode (K=256 per instruction, N=512),
    accumulating in PSUM fp32; pass 0 emitted j-major (x-arrival
    paced), later passes h-major with 8 concurrent PSUM chains so the
    PE consumes w sub-tiles in arrival order.
  - PSUM evicted on the Scalar engine with scale=4.0 (products are
    (+-0.5)^2 = +-0.25) to fp16 output tiles (integers <= 2048 are
    exact in fp16), halving store traffic; host widens to f32.
  - Stores for passes 0/1 queue behind the last input DMA (filling the
    post-input window); pass 2+3 stores merge into one 256KB transfer
    per m-tile right after the final evict.

All arithmetic is exact: +-0.5 exact in fp8e4, products +-0.25 exact,
sums are multiples of 0.25 bounded by 512 (fp32-exact), x4 exact,
results are integers in [-2048, 2048], all exactly representable in
fp16. The host fp16->f32 widening is exact.
"""

import sys

if "/opt/trn_rl_repo" not in sys.path:
    sys.path.insert(0, "/opt/trn_rl_repo")

import numpy as np

B_FULL, D_IN, UNITS = 8192, 2048, 2048
N_CORES = 8
B_CORE = B_FULL // N_CORES  # 1024
P = 128


def build_kernel(B=B_CORE, D=D_IN, U=UNITS, pass_w=512, xgrp=2, wsub=4,
                 use_dr=True, out_dt="float16", in_dt="bfloat16"):
    """Build (and compile) the per-core Bass kernel. Returns the Bacc nc."""
    from concourse import bacc
    import concourse.mybir as mybir
    import concourse.tile as tile

    f32 = mybir.dt.float32
    f8 = mybir.dt.float8e4
    odt = getattr(mybir.dt, out_dt)
    idt = getattr(mybir.dt, in_dt)

    assert B % P == 0 and D % P == 0 and U % pass_w == 0 and pass_w % 512 == 0
    MT = B // P            # m-tiles (8)
    KT = D // P            # k-subtiles (16)
    NQ = U // pass_w       # n passes (4)
    NB = pass_w // 512     # psum banks per (m-tile, pass) (1)
    XG = MT // xgrp        # x^T groups (4)
    step = 2 if use_dr else 1

    # per-pass k-splits (number of k-subtiles per w sub-tile)
    if KT % wsub == 0:
        wsplits = [[wsub] * (KT // wsub)] * NQ
    else:
        wsplits = [[KT]] * NQ
    for sp in wsplits:
        assert all(s % step == 0 for s in sp) and sum(sp) == KT

    nc = bacc.Bacc("TRN2", target_bir_lowering=False)
    x_d = nc.dram_tensor("xT", [D, B], idt, kind="ExternalInput")
    w_d = nc.dram_tensor("w", [D, U], idt, kind="ExternalInput")
    o_d = nc.dram_tensor("out", [B, U], odt, kind="ExternalOutput")

    x_ap = x_d[:].rearrange("(s p) m -> p s m", p=P)       # [128, KT, B]
    w_ap = w_d[:].rearrange("(s p) u -> p s u", p=P)       # [128, KT, U]
    o_ap = o_d[:].rearrange("(j p) u -> j p u", p=P)       # [MT, 128, U]

    GE = mybir.AluOpType.is_ge
    SUB = mybir.AluOpType.subtract

    with tile.TileContext(nc) as tc, \
         tc.tile_pool(name="wstage", bufs=3) as wstage, \
         tc.tile_pool(name="xstage", bufs=2) as xstage, \
         tc.tile_pool(name="resident", bufs=1) as resident, \
         tc.tile_pool(name="mpsum", bufs=8, space="PSUM") as mpsum:

        # separate resident tiles => fine-grained scheduler dependencies
        w8 = [[resident.tile([P, s, pass_w], f8, name=f"w8_{q}_{h}")
               for h, s in enumerate(wsplits[q])] for q in range(NQ)]
        xT8 = [resident.tile([P, KT, xgrp * P], f8, name=f"xT8_{g}")
               for g in range(XG)]
        ost = [resident.tile([P, U], odt, name=f"ost_{j}")
               for j in range(MT)]

        def emit_x_group(g):
            m0 = g * xgrp * P
            xs = xstage.tile([P, KT, xgrp * P], idt, tag="xs")
            nc.sync.dma_start(xs, x_ap[:, :, m0:m0 + xgrp * P])
            nc.vector.tensor_scalar(xT8[g], xs, 0.0, 0.5, GE, SUB)

        def emit_w_subtile(q, h):
            n0 = q * pass_w
            s = wsplits[q][h]
            ks0 = sum(wsplits[q][:h])
            ws = wstage.tile([P, s, pass_w], idt, tag="ws",
                             name=f"ws_{q}_{h}")
            nc.sync.dma_start(ws, w_ap[:, ks0:ks0 + s, n0:n0 + pass_w])
            nc.vector.tensor_scalar(w8[q][h], ws, 0.0, 0.5, GE, SUB)

        psum_tiles = {}

        def emit_mm_chunk(q, j, h):
            g, jo = j // xgrp, (j % xgrp) * P
            if (q, j) not in psum_tiles:
                psum_tiles[(q, j)] = [
                    mpsum.tile([P, 512], f32, tag="ps", name=f"ps_{q}_{j}_{b}")
                    for b in range(NB)]
            pss = psum_tiles[(q, j)]
            ks0 = sum(wsplits[q][:h])
            for kc in range(0, wsplits[q][h], step):
                ks = ks0 + kc
                first = ks == 0
                last = ks + step >= KT
                for b in range(NB):
                    if use_dr:
                        nc.tensor.matmul(
                            pss[b],
                            lhsT=xT8[g][:, ks:ks + 2, jo:jo + P],
                            rhs=w8[q][h][:, kc:kc + 2, 512 * b:512 * (b + 1)],
                            start=first, stop=last,
                            perf_mode=mybir.MatmulPerfMode.DoubleRow,
                        )
                    else:
                        nc.tensor.matmul(
                            pss[b],
                            lhsT=xT8[g][:, ks, jo:jo + P],
                            rhs=w8[q][h][:, kc, 512 * b:512 * (b + 1)],
                            start=first, stop=last,
                        )

        def emit_evict(q, j):
            pss = psum_tiles.pop((q, j))
            for b in range(NB):
                # evict with x4 scale: (+-0.5 * +-0.5) sums -> integer out
                nc.scalar.activation(
                    ost[j][:, q * pass_w + 512 * b:q * pass_w + 512 * (b + 1)],
                    pss[b], mybir.ActivationFunctionType.Copy, scale=4.0,
                )

        def emit_mm(q, j):
            for h in range(len(wsplits[q])):
                emit_mm_chunk(q, j, h)
            emit_evict(q, j)

        def emit_store(j, q, nq=1, engine=None):
            n0 = q * pass_w
            n1 = n0 + nq * pass_w
            (engine or nc.sync).dma_start(
                o_ap[j, :, n0:n1], ost[j][:, n0:n1])

        if (NQ, XG, MT) == (4, 4, 8) and len(wsplits[0]) == 4:
            # Single sync-ring DMA stream: w quarter 0 + x groups woven
            # with pass-0 matmuls, later quarters just-in-time for their
            # (h-major) passes, stores at the back of the FIFO.
            #
            # PE warm-up: dummy DoubleRow matmuls on a zeroed scratch
            # tile fill the initial DMA-latency window so the HAM clock
            # gate reaches 2.4GHz before the first real matmul (its
            # free-running 4096-cycle activity window needs ~3.4us of
            # sustained PE work; results land in a psum slot that is
            # never read).
            warm = resident.tile([P, 2, 512], f8, name="warm")
            nc.gpsimd.memset(warm, 0.0)
            wps = mpsum.tile([P, 512], f32, tag="ps", name="warm_ps")
            N_WARM = 25
            for i in range(N_WARM):
                nc.tensor.matmul(
                    wps, lhsT=warm[:, :, :P], rhs=warm,
                    start=(i == 0), stop=(i == N_WARM - 1),
                    perf_mode=mybir.MatmulPerfMode.DoubleRow,
                )
            emit_w_subtile(0, 0)
            emit_x_group(0)              # m0, m1
            emit_w_subtile(0, 1)
            emit_w_subtile(0, 2)
            emit_mm_chunk(0, 0, 0)
            emit_mm_chunk(0, 1, 0)
            emit_x_group(1)              # m2, m3
            emit_mm_chunk(0, 0, 1)
            emit_mm_chunk(0, 1, 1)
            emit_w_subtile(0, 3)
            emit_mm_chunk(0, 0, 2)
            emit_mm_chunk(0, 1, 2)
            emit_x_group(2)              # m4, m5
            emit_mm(0, 2)
            emit_mm_chunk(0, 0, 3)
            emit_evict(0, 0)
            emit_mm_chunk(0, 1, 3)
            emit_evict(0, 1)
            emit_w_subtile(1, 0)
            emit_mm(0, 3)
            emit_w_subtile(1, 1)
            emit_mm(0, 4)
            emit_x_group(3)              # m6, m7
            emit_mm(0, 5)
            emit_w_subtile(1, 2)
            emit_w_subtile(1, 3)
            # pass 1 h-major for j0..5 (w-arrival paced), weave m6/m7
            for j in range(6):
                emit_mm_chunk(1, j, 0)
            for j in range(6):
                emit_mm_chunk(1, j, 1)
            emit_mm(0, 6)
            for j in range(6):
                emit_mm_chunk(1, j, 2)
            emit_mm(0, 7)
            for j in range(6):
                emit_mm_chunk(1, j, 3)
                emit_evict(1, j)
            for h in range(4):
                emit_w_subtile(2, h)
            emit_mm(1, 6)
            emit_mm(1, 7)
            # pass 2 h-major, all j
            for h in range(4):
                for j in range(MT):
                    emit_mm_chunk(2, j, h)
                    if h == 3:
                        emit_evict(2, j)
            for h in range(4):
                emit_w_subtile(3, h)
            # early stores land in the post-input DMA window
            for j in range(MT):
                emit_store(j, 0)
            for j in range(MT):
                emit_store(j, 1)
            # pass 3 h-major; q2+q3 stores interleaved per-j in the tail
            for h in range(4):
                for j in range(MT):
                    emit_mm_chunk(3, j, h)
                    if h == 3:
                        emit_evict(3, j)
                        emit_store(j, 2, nq=2)
        else:
            # generic fallback (used by small-shape tests)
            for q in range(NQ):
                for h in range(len(wsplits[q])):
                    emit_w_subtile(q, h)
            for g in range(XG):
                emit_x_group(g)
            for q in range(NQ):
                for j in range(MT):
                    emit_mm(q, j)
            for q in range(NQ):
                for j in range(MT):
                    emit_store(j, q)

    nc.compile()
    return nc


_NC_CACHE = {}
LAST_RESULTS = {}


def _get_nc(**kwargs):
    key = tuple(sorted(kwargs.items()))
    if key not in _NC_CACHE:
        _NC_CACHE[key] = build_kernel(**kwargs)
    return _NC_CACHE[key]


def kernel(x, w, _trace=False, _trace_cores=None, **build_kwargs):
    from concourse.bass_utils import run_bass_kernel_spmd

    x = np.asarray(x, dtype=np.float32)
    w = np.asarray(w, dtype=np.float32)
    assert x.shape == (B_FULL, D_IN) and w.shape == (D_IN, UNITS)

    nc = _get_nc(**build_kwargs)
    # Shards ship as bf16: the kernel consumes only input SIGNS, and
    # f32->bf16 rounding preserves the sign of every representable
    # magnitude >= bf16's underflow threshold (~1e-38) -- far below any
    # value the inputs can contain. Binarization itself stays on-device.
    import ml_dtypes
    bf16 = ml_dtypes.bfloat16
    w16 = np.asarray(w, dtype=bf16)
    in_maps = [
        {"xT": np.asarray(x[c * B_CORE:(c + 1) * B_CORE].T, dtype=bf16,
                          order="C"),
         "w": w16}
        for c in range(N_CORES)
    ]
    br = run_bass_kernel_spmd(
        nc, in_maps, list(range(N_CORES)),
        trace=_trace, trace_cores=_trace_cores,
    )
    LAST_RESULTS["br"] = br
    out = np.concatenate(
        [br.results[c]["out"].astype(np.float32) for c in range(N_CORES)],
        axis=0,
    )
    return out


if __name__ == "__main__":
    rng = np.random.default_rng(0)
    x = rng.standard_normal((B_FULL, D_IN), dtype=np.float32)
    w = (rng.standard_normal((D_IN, UNITS), dtype=np.float32) * 0.1).astype(
        np.float32
    )
    out = kernel(x, w)
    exp = np.sign(x + (x == 0)) @ np.sign(w + (w == 0))
    print("max abs err:", np.max(np.abs(out - exp)))


# revision 31
# speedup vs baseline: 1.0289x; 1.0289x over previous
"""BinaryDense kernel for Trainium2: out = sign(x) @ sign(w).

x: [8192, 2048] f32, w: [2048, 2048] f32 -> out: [8192, 2048] f32.

Strategy: data-parallel shard of the batch dim across 8 NeuronCores
(1024 rows each, w replicated). The host hands each core its x shard
pre-transposed (layout choice for the shard) and both inputs as bf16
(sign-preserving for every magnitude the f32 inputs can contain --
bf16 underflows at ~1e-38; binarization itself stays on-device), which
halves input DMA to 12MB/core. Per core:
  - w streamed in (pass, k-sub-tile) 0.5MB slices, binarized on DVE to
    fp8e4 (+-0.5 via one tensor_scalar: (w >= 0) - 0.5) into separate
    small resident tiles (separate tiles => fine-grained scheduler
    dependencies, so matmuls start as soon as their slice lands).
  - x^T streamed in m-pair column groups [128, 16, 256], binarized the
    same way.
  - Matmuls in fp8 DoubleRow mode (K=256 per instruction, N=512),
    accumulating in PSUM fp32; pass 0 emitted j-major (x-arrival
    paced), later passes h-major with 8 concurrent PSUM chains so the
    PE consumes w sub-tiles in arrival order.
  - PSUM evicted on the Scalar engine with scale=4.0 (products are
    (+-0.5)^2 = +-0.25) to fp16 output tiles (integers <= 2048 are
    exact in fp16), halving store traffic; host widens to f32.
  - Stores for passes 0/1 queue behind the last input DMA (filling the
    post-input window); pass 2+3 stores merge into one 256KB transfer
    per m-tile right after the final evict.

All arithmetic is exact: +-0.5 exact in fp8e4, products +-0.25 exact,
sums are multiples of 0.25 bounded by 512 (fp32-exact), x4 exact,
results are integers in [-2048, 2048], all exactly representable in
fp16. The host fp16->f32 widening is exact.
"""

import sys

if "/opt/trn_rl_repo" not in sys.path:
    sys.path.insert(0, "/opt/trn_rl_repo")

import numpy as np

B_FULL, D_IN, UNITS = 8192, 2048, 2048
N_CORES = 8
B_CORE = B_FULL // N_CORES  # 1024
P = 128


def build_kernel(B=B_CORE, D=D_IN, U=UNITS, pass_w=512, xgrp=2, wsub=4,
                 use_dr=True, out_dt="float16", in_dt="bfloat16"):
    """Build (and compile) the per-core Bass kernel. Returns the Bacc nc."""
    from concourse import bacc
    import concourse.mybir as mybir
    import concourse.tile as tile

    f32 = mybir.dt.float32
    f8 = mybir.dt.float8e4
    odt = getattr(mybir.dt, out_dt)
    idt = getattr(mybir.dt, in_dt)

    assert B % P == 0 and D % P == 0 and U % pass_w == 0 and pass_w % 512 == 0
    MT = B // P            # m-tiles (8)
    KT = D // P            # k-subtiles (16)
    NQ = U // pass_w       # n passes (4)
    NB = pass_w // 512     # psum banks per (m-tile, pass) (1)
    XG = MT // xgrp        # x^T groups (4)
    step = 2 if use_dr else 1

    # per-pass k-splits (number of k-subtiles per w sub-tile)
    if KT % wsub == 0:
        wsplits = [[wsub] * (KT // wsub)] * NQ
    else:
        wsplits = [[KT]] * NQ
    for sp in wsplits:
        assert all(s % step == 0 for s in sp) and sum(sp) == KT

    nc = bacc.Bacc("TRN2", target_bir_lowering=False)
    x_d = nc.dram_tensor("xT", [D, B], idt, kind="ExternalInput")
    w_d = nc.dram_tensor("w", [D, U], idt, kind="ExternalInput")
    o_d = nc.dram_tensor("out", [B, U], odt, kind="ExternalOutput")

    x_ap = x_d[:].rearrange("(s p) m -> p s m", p=P)       # [128, KT, B]
    w_ap = w_d[:].rearrange("(s p) u -> p s u", p=P)       # [128, KT, U]
    o_ap = o_d[:].rearrange("(j p) u -> j p u", p=P)       # [MT, 128, U]

    GE = mybir.AluOpType.is_ge
    SUB = mybir.AluOpType.subtract

    with tile.TileContext(nc) as tc, \
         tc.tile_pool(name="wstage", bufs=3) as wstage, \
         tc.tile_pool(name="xstage", bufs=2) as xstage, \
         tc.tile_pool(name="resident", bufs=1) as resident, \
         tc.tile_pool(name="mpsum", bufs=8, space="PSUM") as mpsum:

        # separate resident tiles => fine-grained scheduler dependencies
        w8 = [[resident.tile([P, s, pass_w], f8, name=f"w8_{q}_{h}")
               for h, s in enumerate(wsplits[q])] for q in range(NQ)]
        xT8 = [resident.tile([P, KT, xgrp * P], f8, name=f"xT8_{g}")
               for g in range(XG)]
        ost = [resident.tile([P, U], odt, name=f"ost_{j}")
               for j in range(MT)]

        def emit_x_group(g):
            m0 = g * xgrp * P
            xs = xstage.tile([P, KT, xgrp * P], idt, tag="xs")
            nc.sync.dma_start(xs, x_ap[:, :, m0:m0 + xgrp * P])
            nc.vector.tensor_scalar(xT8[g], xs, 0.0, 0.5, GE, SUB)

        def emit_w_subtile(q, h):
            n0 = q * pass_w
            s = wsplits[q][h]
            ks0 = sum(wsplits[q][:h])
            ws = wstage.tile([P, s, pass_w], idt, tag="ws",
                             name=f"ws_{q}_{h}")
            nc.sync.dma_start(ws, w_ap[:, ks0:ks0 + s, n0:n0 + pass_w])
            nc.vector.tensor_scalar(w8[q][h], ws, 0.0, 0.5, GE, SUB)

        psum_tiles = {}

        def emit_mm_chunk(q, j, h):
            g, jo = j // xgrp, (j % xgrp) * P
            if (q, j) not in psum_tiles:
                psum_tiles[(q, j)] = [
                    mpsum.tile([P, 512], f32, tag="ps", name=f"ps_{q}_{j}_{b}")
                    for b in range(NB)]
            pss = psum_tiles[(q, j)]
            ks0 = sum(wsplits[q][:h])
            for kc in range(0, wsplits[q][h], step):
                ks = ks0 + kc
                first = ks == 0
                last = ks + step >= KT
                for b in range(NB):
                    if use_dr:
                        nc.tensor.matmul(
                            pss[b],
                            lhsT=xT8[g][:, ks:ks + 2, jo:jo + P],
                            rhs=w8[q][h][:, kc:kc + 2, 512 * b:512 * (b + 1)],
                            start=first, stop=last,
                            perf_mode=mybir.MatmulPerfMode.DoubleRow,
                        )
                    else:
                        nc.tensor.matmul(
                            pss[b],
                            lhsT=xT8[g][:, ks, jo:jo + P],
                            rhs=w8[q][h][:, kc, 512 * b:512 * (b + 1)],
                            start=first, stop=last,
                        )

        def emit_evict(q, j):
            pss = psum_tiles.pop((q, j))
            for b in range(NB):
                # evict with x4 scale: (+-0.5 * +-0.5) sums -> integer out
                nc.scalar.activation(
                    ost[j][:, q * pass_w + 512 * b:q * pass_w + 512 * (b + 1)],
                    pss[b], mybir.ActivationFunctionType.Copy, scale=4.0,
                )

        def emit_mm(q, j):
            for h in range(len(wsplits[q])):
                emit_mm_chunk(q, j, h)
            emit_evict(q, j)

        def emit_store(j, q, nq=1, engine=None):
            n0 = q * pass_w
            n1 = n0 + nq * pass_w
            (engine or nc.sync).dma_start(
                o_ap[j, :, n0:n1], ost[j][:, n0:n1])

        if (NQ, XG, MT) == (4, 4, 8) and len(wsplits[0]) == 4:
            # Single sync-ring DMA stream: w quarter 0 + x groups woven
            # with pass-0 matmuls, later quarters just-in-time for their
            # (h-major) passes, stores at the back of the FIFO.
            emit_w_subtile(0, 0)
            emit_x_group(0)              # m0, m1
            emit_w_subtile(0, 1)
            emit_w_subtile(0, 2)
            emit_mm_chunk(0, 0, 0)
            emit_mm_chunk(0, 1, 0)
            emit_x_group(1)              # m2, m3
            emit_mm_chunk(0, 0, 1)
            emit_mm_chunk(0, 1, 1)
            emit_w_subtile(0, 3)
            emit_mm_chunk(0, 0, 2)
            emit_mm_chunk(0, 1, 2)
            emit_x_group(2)              # m4, m5
            emit_mm(0, 2)
            emit_mm_chunk(0, 0, 3)
            emit_evict(0, 0)
            emit_mm_chunk(0, 1, 3)
            emit_evict(0, 1)
            emit_w_subtile(1, 0)
            emit_mm(0, 3)
            emit_w_subtile(1, 1)
            emit_mm(0, 4)
            emit_x_group(3)              # m6, m7
            emit_mm(0, 5)
            emit_w_subtile(1, 2)
            emit_w_subtile(1, 3)
            # pass 1 h-major for j0..5 (w-arrival paced), weave m6/m7
            for j in range(6):
                emit_mm_chunk(1, j, 0)
            for j in range(6):
                emit_mm_chunk(1, j, 1)
            emit_mm(0, 6)
            for j in range(6):
                emit_mm_chunk(1, j, 2)
            emit_mm(0, 7)
            for j in range(6):
                emit_mm_chunk(1, j, 3)
                emit_evict(1, j)
            for h in range(4):
                emit_w_subtile(2, h)
            emit_mm(1, 6)
            emit_mm(1, 7)
            # pass 2 h-major, all j
            for h in range(4):
                for j in range(MT):
                    emit_mm_chunk(2, j, h)
                    if h == 3:
                        emit_evict(2, j)
            for h in range(4):
                emit_w_subtile(3, h)
            # early stores land in the post-input DMA window
            for j in range(MT):
                emit_store(j, 0)
            for j in range(MT):
                emit_store(j, 1)
            # pass 3 h-major; q2+q3 stores interleaved per-j in the tail
            for h in range(4):
                for j in range(MT):
                    emit_mm_chunk(3, j, h)
                    if h == 3:
                        emit_evict(3, j)
                        emit_store(j, 2, nq=2)
        else:
            # generic fallback (used by small-shape tests)
            for q in range(NQ):
                for h in range(len(wsplits[q])):
                    emit_w_subtile(q, h)
            for g in range(XG):
                emit_x_group(g)
            for q in range(NQ):
                for j in range(MT):
                    emit_mm(q, j)
            for q in range(NQ):
                for j in range(MT):
                    emit_store(j, q)

    nc.compile()
    return nc


_NC_CACHE = {}
LAST_RESULTS = {}


def _get_nc(**kwargs):
    key = tuple(sorted(kwargs.items()))
    if key not in _NC_CACHE:
        _NC_CACHE[key] = build_kernel(**kwargs)
    return _NC_CACHE[key]


def kernel(x, w, _trace=False, _trace_cores=None, **build_kwargs):
    from concourse.bass_utils import run_bass_kernel_spmd

    x = np.asarray(x, dtype=np.float32)
    w = np.asarray(w, dtype=np.float32)
    assert x.shape == (B_FULL, D_IN) and w.shape == (D_IN, UNITS)

    nc = _get_nc(**build_kwargs)
    # Shards ship as bf16: the kernel consumes only input SIGNS, and
    # f32->bf16 rounding preserves the sign of every representable
    # magnitude >= bf16's underflow threshold (~1e-38) -- far below any
    # value the inputs can contain. Binarization itself stays on-device.
    import ml_dtypes
    bf16 = ml_dtypes.bfloat16
    w16 = np.asarray(w, dtype=bf16)
    in_maps = [
        {"xT": np.asarray(x[c * B_CORE:(c + 1) * B_CORE].T, dtype=bf16,
                          order="C"),
         "w": w16}
        for c in range(N_CORES)
    ]
    br = run_bass_kernel_spmd(
        nc, in_maps, list(range(N_CORES)),
        trace=_trace, trace_cores=_trace_cores,
    )
    LAST_RESULTS["br"] = br
    out = np.concatenate(
        [br.results[c]["out"].astype(np.float32) for c in range(N_CORES)],
        axis=0,
    )
    return out


if __name__ == "__main__":
    rng = np.random.default_rng(0)
    x = rng.standard_normal((B_FULL, D_IN), dtype=np.float32)
    w = (rng.standard_normal((D_IN, UNITS), dtype=np.float32) * 0.1).astype(
        np.float32
    )
    out = kernel(x, w)
    exp = np.sign(x + (x == 0)) @ np.sign(w + (w == 0))
    print("max abs err:", np.max(np.abs(out - exp)))
